# revision 23
# baseline (speedup 1.0000x reference)
"""GAT (2-layer, 4-head) full-graph kernel for 8 Trainium2 NeuronCores.

Strategy
--------
Nodes are renumbered by in-degree and dealt to the 8 cores so every core
owns 6272 destination nodes (49 tiles of 128) with a degree profile that
matches the other cores tile-for-tile.  All segment reductions (softmax
denominator and the weighted feature aggregation) are local to the core
that owns the destination node.

Layer 1's feature table (feat = x @ W1 plus the attention projections) is
built REPLICATED on every core straight from the (replicated) input x —
no collective.  Layer 2 needs one AllGather of the per-node layer-1
output rows, produced tile-by-tile inside the layer-1 edge loop so the
collective starts immediately after the last tile.

The int16 gather indices force a lo/hi table split at 32768.  The lo
window [0, 32768) and hi window [17408, 50176) OVERLAP by 15360 rows;
sources placed in the overlap can be addressed from either half.  The
host renumbering (a) parks the highest out-degree nodes in the overlap
and (b) greedily assigns the rest so each destination's in-edges split
near-evenly between the halves, which shrinks the padded-CSR slot count
from 1.74x the edge count to ~1.04x.

Edge-slot padding points at a dedicated dummy table row (global row
32767, reachable from both windows) whose el is -150, so padded slots
contribute exp(leaky(-150+er)) ~ 1e-13 to numerator and denominator and
no masking is needed.
"""

import sys

sys.path.insert(0, "/opt/trn_rl_repo")

import numpy as np
import ml_dtypes

import concourse.bass as bass
import concourse.bacc as bacc
import concourse.mybir as mybir
from concourse import tile as tile_mod
from concourse import library_config

# ---------------------------------------------------------------- constants
N = 50000
E = 800000
H = 4
D = 32
HD = H * D          # 128
IN_F = 128
NEG_SLOPE = 0.2

NCORES = 8
NT = 49             # dst tiles per core
NPC = NT * 128      # 6272 nodes per core
NPRIME = NCORES * NPC   # 50176 table rows
LOSPLIT = 32768
HI_OFF = NPRIME - LOSPLIT   # 17408; flex window = [HI_OFF, LOSPLIT)
DUMLOC = 1407       # local pad slot on every core; global row 5*6272+1407=32767
DUMROW = 5 * NPC + DUMLOC   # 32767 == LOSPLIT-1: reachable from both halves
DUMMY_LO = DUMROW           # 32767
DUMMY_HI = DUMROW - HI_OFF  # 15359
EL_DUMMY = -150.0

CAP_LO = 28         # max lo columns per gather chunk
CAP_HI = 28
TGROUP = 14         # tiles per DMA group in the table1 build (392 = 28*14)

F32 = mybir.dt.float32
BF16 = mybir.dt.bfloat16
I16 = mybir.dt.int16
AL = mybir.AluOpType
AF = mybir.ActivationFunctionType


# ---------------------------------------------------------------- host plan
def _balanced_cores(order, deg_in, deg_out, dst_by_src, optr):
    """Assign each in-degree rank a core so that (a) high out-degree nodes
    land in the lo/hi overlap window and (b) every destination's in-edges
    split near-evenly between the lo and hi index windows."""
    nblocks = (N + NCORES - 1) // NCORES
    tile_of_rank = np.arange(N) // (NCORES * 128)
    K_t = np.array([deg_in[order[t * 1024:(t + 1) * 1024]].max()
                    for t in range(NT)])
    rank_of = np.empty(N, np.int64)
    rank_of[order] = np.arange(N)
    tile_of_node = tile_of_rank[rank_of]

    blocks = []
    is_flex_node = np.zeros(N, bool)
    for q in range(nblocks):
        r0 = q * NCORES
        blk = order[r0:min(N, r0 + NCORES)]
        flex = [3, 4]
        if q >= 4863:
            flex.append(2)
        if q <= 1406:
            flex.append(5)
        lo_slots = [c for c in (0, 1, 2) if c not in flex]
        hi_slots = [c for c in (5, 6, 7) if c not in flex]
        nodes = blk[np.argsort(-deg_out[blk], kind="stable")]
        is_flex_node[nodes[:len(flex)]] = True
        blocks.append((flex, lo_slots, hi_slots, nodes, blk, r0))

    # flex in-edge count per dst
    f_cnt = np.bincount(
        dst_by_src[np.repeat(is_flex_node, np.diff(optr))], minlength=N)
    slack = (K_t[tile_of_node] - deg_in) + f_cnt

    imb = np.zeros(N, np.int64)
    side = np.zeros(N, np.int8)
    for sweep in range(3):
        changed = 0
        for flex, lo_slots, hi_slots, nodes, blk, r0 in blocks:
            rest = nodes[len(flex):]
            if len(rest) == 0:
                continue
            for v in rest:
                s = side[v]
                if s:
                    us = dst_by_src[optr[v]:optr[v + 1]]
                    np.add.at(imb, us, -int(s))
            deltas = np.empty(len(rest))
            for i, v in enumerate(rest):
                us = dst_by_src[optr[v]:optr[v + 1]]
                iu = imb[us]
                su = slack[us]
                b0 = np.maximum(0, np.abs(iu) - su)
                dlo = (np.maximum(0, np.abs(iu + 1) - su) - b0).sum()
                dhi = (np.maximum(0, np.abs(iu - 1) - su) - b0).sum()
                sgn = np.sign(iu).sum()
                deltas[i] = (dlo + 0.001 * sgn) - (dhi - 0.001 * sgn)
            pidx = np.argsort(deltas, kind="stable")
            nl = len(lo_slots)
            for j, v in enumerate(rest[pidx]):
                s = 1 if j < nl else -1
                if side[v] != s:
                    changed += 1
                side[v] = s
                us = dst_by_src[optr[v]:optr[v + 1]]
                np.add.at(imb, us, int(s))
        if changed == 0:
            break

    core_of_rank = np.empty(N, np.int64)
    for flex, lo_slots, hi_slots, nodes, blk, r0 in blocks:
        rankpos = {int(v): r0 + j for j, v in enumerate(blk)}
        for c, v in zip(flex, nodes[:len(flex)]):
            core_of_rank[rankpos[int(v)]] = c
        rest = nodes[len(flex):]
        lo_nodes = [v for v in rest if side[v] == 1]
        hi_nodes = [v for v in rest if side[v] == -1]
        for c, v in zip(lo_slots, lo_nodes):
            core_of_rank[rankpos[int(v)]] = c
        for c, v in zip(hi_slots, hi_nodes):
            core_of_rank[rankpos[int(v)]] = c
    return core_of_rank


def _plan(src, dst):
    """Graph preprocessing: balanced node renumbering + padded-CSR layout."""
    deg_in = np.bincount(dst, minlength=N)
    deg_out = np.bincount(src, minlength=N)
    order = np.argsort(-deg_in, kind="stable")          # rank -> old id

    eorder = np.argsort(src, kind="stable")
    dst_by_src = dst[eorder]
    optr = np.concatenate([[0], np.cumsum(deg_out)])

    core_of_rank = _balanced_cores(order, deg_in, deg_out, dst_by_src, optr)

    seq = np.arange(N) // NCORES
    local = seq + (seq >= DUMLOC)                    # skip the dummy slot
    newid = np.empty(N, np.int64)
    newid[order] = core_of_rank * NPC + local

    s2 = newid[src]
    d2 = newid[dst]

    # per-edge side: strict windows forced, flex edges balance each dst
    strict_lo = s2 < HI_OFF
    strict_hi = s2 >= LOSPLIT
    flex_e = ~strict_lo & ~strict_hi
    a = np.bincount(d2[strict_lo], minlength=NPRIME)
    b = np.bincount(d2[strict_hi], minlength=NPRIME)
    f = np.bincount(d2[flex_e], minlength=NPRIME)
    d = a + b + f
    x = np.clip((d + 1) // 2 - a, 0, f)              # flex edges sent to lo

    idxs = np.flatnonzero(flex_e)
    se = idxs[np.argsort(d2[idxs], kind="stable")]
    grp = d2[se]
    first = np.r_[True, grp[1:] != grp[:-1]]
    runstart = np.flatnonzero(first)
    runid = np.cumsum(first) - 1
    occ = np.arange(len(se)) - runstart[runid]
    flex_lo = np.zeros(E, bool)
    flex_lo[se] = occ < x[grp]
    islo = strict_lo | (flex_e & flex_lo)

    # per-(new dst, side) counts -> common per-tile column counts
    cnt_lo = np.bincount(d2[islo], minlength=NPRIME).reshape(NCORES, NT, 128)
    cnt_hi = np.bincount(d2[~islo], minlength=NPRIME).reshape(NCORES, NT, 128)
    K_lo = np.maximum(cnt_lo.max(axis=(0, 2)), 1).astype(np.int64)
    K_hi = np.maximum(cnt_hi.max(axis=(0, 2)), 1).astype(np.int64)

    # occurrence index of each edge within its (dst, side) group
    okey = d2 * 2 + (~islo)
    sorted_e = np.argsort(okey, kind="stable")
    ok_sorted = okey[sorted_e]
    first = np.r_[True, ok_sorted[1:] != ok_sorted[:-1]]
    runid = np.cumsum(first) - 1
    runstart = np.flatnonzero(first)
    k = np.empty(E, np.int64)
    k[sorted_e] = np.arange(E) - runstart[runid]

    ecore = d2 // NPC
    locd = d2 % NPC
    tl = locd // 128
    pt = locd % 128

    off_lo = np.concatenate([[0], np.cumsum(K_lo)])  # lo col offset per tile
    off_hi = np.concatenate([[0], np.cumsum(K_hi)])
    SLO = int(off_lo[-1])
    SHI = int(off_hi[-1])

    idx_lo = np.full((NCORES, SLO * 128), DUMMY_LO, np.int32)
    idx_hi = np.full((NCORES, SHI * 128), DUMMY_HI, np.int32)
    sel = islo
    slot = (off_lo[tl[sel]] + k[sel]) * 128 + pt[sel]
    idx_lo[ecore[sel], slot] = s2[sel]
    sel = ~islo
    slot = (off_hi[tl[sel]] + k[sel]) * 128 + pt[sel]
    idx_hi[ecore[sel], slot] = s2[sel] - HI_OFF

    assert idx_lo.max() < LOSPLIT and idx_hi.max() < LOSPLIT
    assert idx_lo.min() >= 0 and idx_hi.min() >= 0

    def pack(a):  # [S*128] -> [128, S*8] int16 (16-wrap, replicated x8)
        return np.ascontiguousarray(
            np.tile(a.reshape(-1, 16).T, (8, 1)).astype(np.int16)
        )

    idx_lo_pk = np.stack([pack(idx_lo[c]) for c in range(NCORES)])
    idx_hi_pk = np.stack([pack(idx_hi[c]) for c in range(NCORES)])

    return dict(
        newid=newid, K_lo=K_lo, K_hi=K_hi,
        chunks=_make_chunks(K_lo, K_hi, CAP_LO, CAP_HI),
        SLO=SLO, SHI=SHI, idx_lo=idx_lo_pk, idx_hi=idx_hi_pk,
    )


def _make_chunks(K_lo, K_hi, cap_lo, cap_hi):
    chunks = []  # (t0, t1, LO0, HI0, clo, chi)
    t0, LO0, HI0, clo, chi = 0, 0, 0, 0, 0
    for t in range(NT):
        if t > t0 and (clo + K_lo[t] > cap_lo or chi + K_hi[t] > cap_hi):
            chunks.append((t0, t, LO0, HI0, clo, chi))
            LO0 += clo
            HI0 += chi
            t0, clo, chi = t, 0, 0
        clo += int(K_lo[t])
        chi += int(K_hi[t])
    chunks.append((t0, NT, LO0, HI0, clo, chi))
    return chunks


# ---------------------------------------------------------- program builder
def _build_program(plan, reps=1, ablate=(), act_expand=True,
                   gather_queues=4, single_packet=False, nqueues=None,
                   gbufs=5):
    K_lo, K_hi = plan["K_lo"], plan["K_hi"]
    SLO, SHI = plan["SLO"], plan["SHI"]
    NG = NCORES * NT // TGROUP          # table1 build groups (28)

    nc = bacc.Bacc(None, target_bir_lowering=False, debug=False,
                   num_swdge_queues=min(4, nqueues or max(4, gather_queues)))

    xT_full = nc.declare_dram_parameter("xT_full", [128, NPRIME], BF16,
                                        isOutput=False)
    xT_own = nc.declare_dram_parameter("xT_own", [128, NPC], BF16,
                                       isOutput=False)
    w1ext_in = nc.declare_dram_parameter("w1ext", [128, 132], BF16,
                                         isOutput=False)
    w1ar_in = nc.declare_dram_parameter("w1ar", [128, 4], BF16,
                                        isOutput=False)
    w2ext_in = nc.declare_dram_parameter("w2ext", [128, 136], BF16,
                                         isOutput=False)
    b1b_in = nc.declare_dram_parameter("b1b", [128, 128], F32, isOutput=False)
    b2b_in = nc.declare_dram_parameter("b2b", [128, 128], F32, isOutput=False)
    dum_in = nc.declare_dram_parameter("dumrow", [1, 256], BF16,
                                       isOutput=False)
    ident_in = nc.declare_dram_parameter("ident", [128, 128], BF16,
                                         isOutput=False)
    idxlo_in = nc.declare_dram_parameter("idxlo", [128, SLO * 8], I16,
                                         isOutput=False)
    idxhi_in = nc.declare_dram_parameter("idxhi", [128, SHI * 8], I16,
                                         isOutput=False)
    y_out = nc.declare_dram_parameter("y", [NPC, 32], F32, isOutput=True)

    table1 = nc.dram_tensor("table1", [NPRIME, 256], BF16)
    shard2 = nc.dram_tensor("shard2", [NPC, 256], BF16)
    table2 = nc.dram_tensor("table2", [NPRIME, 256], BF16,
                            addr_space="Shared")

    with tile_mod.TileContext(nc) as tc:
        nc.gpsimd.load_library(library_config.mlp)
        with (
            tc.tile_pool(name="const", bufs=1) as cp,
            tc.tile_pool(name="mmin", bufs=3) as mp,
            tc.tile_pool(name="rows", bufs=3) as rp,
            tc.tile_pool(name="psum", bufs=2, space="PSUM") as pp,
            tc.tile_pool(name="g", bufs=gbufs) as gp,
            tc.tile_pool(name="wk", bufs=2) as wk,
            tc.tile_pool(name="sm", bufs=4) as sm,
        ):
            # ---- constants into SBUF
            w1ext = cp.tile([128, 132], BF16)
            w1ar = cp.tile([128, 4], BF16)
            w2ext = cp.tile([128, 136], BF16)
            b1b = cp.tile([128, 128], F32)
            b2b = cp.tile([128, 128], F32)
            ident = cp.tile([128, 128], BF16)
            idxlo = cp.tile([128, SLO * 8], I16)
            idxhi = cp.tile([128, SHI * 8], I16)
            drow = cp.tile([1, 256], BF16)
            er1 = cp.tile([128, NT * 4], F32)
            er2 = cp.tile([128, NT * 4], F32)
            h_buf = cp.tile([128, NT * 128], BF16)

            nc.sync.dma_start(out=w1ext[:], in_=w1ext_in[:])
            nc.sync.dma_start(out=w1ar[:], in_=w1ar_in[:])
            nc.sync.dma_start(out=w2ext[:], in_=w2ext_in[:])
            nc.sync.dma_start(out=b1b[:], in_=b1b_in[:])
            nc.sync.dma_start(out=b2b[:], in_=b2b_in[:])
            nc.sync.dma_start(out=ident[:], in_=ident_in[:])
            nc.sync.dma_start(out=idxlo[:], in_=idxlo_in[:])
            nc.sync.dma_start(out=idxhi[:], in_=idxhi_in[:])
            nc.sync.dma_start(out=drow[:], in_=dum_in[:])

            # ---- replicated table1 build: all 50176 rows on every core
            def table1_build():
              for g in range(NG):
                lh = mp.tile([128, TGROUP * 128], BF16, tag="lh")
                nc.sync.dma_start(
                    out=lh[:],
                    in_=xT_full[:, g * TGROUP * 128:(g + 1) * TGROUP * 128])
                row = rp.tile([128, TGROUP, 132], BF16, tag="row")
                for j in range(TGROUP):
                    ps = pp.tile([128, 132], F32, tag="mm")
                    nc.tensor.matmul(ps[:],
                                     lhsT=lh[:, j * 128:(j + 1) * 128],
                                     rhs=w1ext[:], start=True, stop=True)
                    nc.scalar.activation(row[:, j, :], ps[:], AF.Copy)
                out_ap = table1[g * TGROUP * 128:(g + 1) * TGROUP * 128,
                                0:132].rearrange("(j p) c -> p j c", p=128)
                nc.sync.dma_start(out=out_ap, in_=row[:])
              # dummy row patch (el = -150)
              nc.sync.dma_start(out=table1[DUMROW:DUMROW + 1, 0:132],
                                in_=drow[:, 0:132])

            # ---- er1 for own dst nodes: (x_own @ W1) @ ar1  (per tile)
            def er1_build():
              for g in range(7):
                lh = mp.tile([128, 7 * 128], BF16, tag="lh7")
                nc.sync.dma_start(
                    out=lh[:], in_=xT_own[:, g * 896:(g + 1) * 896])
                for j in range(7):
                    t = g * 7 + j
                    ps4 = pp.tile([128, 4], F32, tag="ps4")
                    nc.tensor.matmul(ps4[:],
                                     lhsT=lh[:, j * 128:(j + 1) * 128],
                                     rhs=w1ar[:], start=True, stop=True)
                    nc.vector.tensor_copy(er1[:, t * 4:(t + 1) * 4], ps4[:])

            # ---- edge phase for one layer (one merged gather pair
            #      per dst tile; lo slots then hi slots, contiguous)
            off_lo = [0]
            off_hi = [0]
            for t in range(NT):
                off_lo.append(off_lo[-1] + int(K_lo[t]))
                off_hi.append(off_hi[-1] + int(K_hi[t]))
            MAXKT = max(int(K_lo[t]) + int(K_hi[t]) for t in range(NT))

            def edge_phase(table, er_t, layer):
                for t in range(NT):
                    KL, KH = int(K_lo[t]), int(K_hi[t])
                    KT = KL + KH
                    LO0, HI0 = off_lo[t], off_hi[t]
                    g = gp.tile([128, MAXKT, 256], BF16, tag="g")
                    if "halfrow" in ablate:
                        gh = g.rearrange("p c e -> p (c e)")[
                            :, 0:KT * 128].rearrange("p (c e) -> p c e",
                                                     e=128)
                        nc.gpsimd.dma_gather(
                            out_ap=gh[:, 0:KL, :],
                            in_ap=table[0:LOSPLIT, 0:128],
                            idxs_ap=idxlo[:, LO0 * 8:(LO0 + KL) * 8],
                            num_idxs=KL * 128, num_idxs_reg=KL * 128,
                            elem_size=128, elem_step=256,
                            single_packet=single_packet, queue_num=0,
                        )
                        nc.gpsimd.dma_gather(
                            out_ap=gh[:, KL:KT, :],
                            in_ap=table[HI_OFF:NPRIME, 0:128],
                            idxs_ap=idxhi[:, HI0 * 8:(HI0 + KH) * 8],
                            num_idxs=KH * 128, num_idxs_reg=KH * 128,
                            elem_size=128, elem_step=256,
                            single_packet=single_packet, queue_num=0,
                        )
                    elif "nogather" not in ablate:
                        nc.gpsimd.dma_gather(
                            out_ap=g[:, 0:KL, :],
                            in_ap=table[0:LOSPLIT, :],
                            idxs_ap=idxlo[:, LO0 * 8:(LO0 + KL) * 8],
                            num_idxs=KL * 128, num_idxs_reg=KL * 128,
                            elem_size=256, single_packet=single_packet,
                            queue_num=0,
                        )
                        nc.gpsimd.dma_gather(
                            out_ap=g[:, KL:KT, :],
                            in_ap=table[HI_OFF:NPRIME, :],
                            idxs_ap=idxhi[:, HI0 * 8:(HI0 + KH) * 8],
                            num_idxs=KH * 128, num_idxs_reg=KH * 128,
                            elem_size=256, single_packet=single_packet,
                            queue_num=0,
                        )
                    else:
                        nc.vector.memset(g[:, 0:1, 0:4], 0.0)

                    def shard2_row(t):
                        psT = pp.tile([128, 128], BF16, tag="psT")
                        nc.tensor.transpose(
                            psT[:], h_buf[:, t * 128:(t + 1) * 128],
                            ident[:])
                        lh2 = mp.tile([128, 128], BF16, tag="lh2")
                        nc.scalar.activation(lh2[:], psT[:], AF.Copy)
                        ps2 = pp.tile([128, 136], F32, tag="mm")
                        nc.tensor.matmul(ps2[:], lhsT=lh2[:], rhs=w2ext[:],
                                         start=True, stop=True)
                        row2 = rp.tile([128, 136], BF16, tag="row2")
                        nc.scalar.activation(row2[:], ps2[:], AF.Copy)
                        nc.vector.tensor_copy(
                            er2[:, t * 4:(t + 1) * 4], ps2[:, 132:136])
                        nc.sync.dma_start(
                            out=shard2[t * 128:(t + 1) * 128, 0:136],
                            in_=row2[:])

                    if "noedge" in ablate:
                        dmy = sm.tile([128, 4], F32, tag="dmy")
                        nc.vector.tensor_tensor(
                            out=dmy[:], in0=g[:, 0, 0:4],
                            in1=er_t[:, 0:4], op=AL.add)
                        if layer == 1:
                            shard2_row(t)
                        continue

                    er_ap = er_t[:, t * 4:(t + 1) * 4].unsqueeze(1)
                    q_t = wk.tile([128, KT, 4], F32, tag="q")
                    nc.vector.tensor_tensor(
                        out=q_t[:],
                        in0=g[:, 0:KT, 128:132],
                        in1=er_ap.broadcast_to([128, KT, 4]),
                        op=AL.add,
                    )
                    e_t = wk.tile([128, KT, 4], F32, tag="e")
                    nc.vector.tensor_scalar_mul(e_t[:], q_t[:], NEG_SLOPE)
                    nc.vector.tensor_tensor(out=e_t[:], in0=e_t[:],
                                            in1=q_t[:], op=AL.max)
                    # exp + broadcast over the 32 feature cols of each head
                    # in one ACT op -> bf16 p-tilde [128, KT, 128]
                    pbig = wk.tile([128, KT, 128], BF16, tag="pbig")
                    nc.scalar.activation(
                        pbig.rearrange("p c (h d) -> p c h d", h=4),
                        e_t[:].unsqueeze(3).broadcast_to([128, KT, 4, 32]),
                        AF.Exp)
                    dn = sm.tile([128, 4], F32, tag="dn")
                    nc.vector.reduce_sum(
                        out=dn[:],
                        in_=pbig.rearrange(
                            "p c (h d) -> p h d c", h=4)[:, :, 0, :],
                        axis=mybir.AxisListType.X)
                    rc = sm.tile([128, 4], F32, tag="rc")
                    nc.vector.reciprocal(rc[:], dn[:])

                    w_t = wk.tile([128, KT, 128], BF16, tag="w")
                    nc.vector.tensor_tensor(
                        out=w_t[:],
                        in0=g[:, 0:KT, 0:128],
                        in1=pbig[:],
                        op=AL.mult,
                    )
                    rst = sm.tile([128, 4, 32], F32, tag="rst")
                    nc.vector.reduce_sum(
                        out=rst[:],
                        in_=w_t.rearrange("p c (h d) -> p h d c", h=4),
                        axis=mybir.AxisListType.X,
                    )
                    o_t = sm.tile([128, 4, 32], F32, tag="o")
                    nc.vector.tensor_tensor(
                        out=o_t[:], in0=rst[:],
                        in1=rc[:].unsqueeze(2).broadcast_to([128, 4, 32]),
                        op=AL.mult,
                    )
                    flat_o = o_t.rearrange("p h d -> p (h d)")
                    if layer == 1:
                        nc.vector.tensor_tensor(
                            out=flat_o, in0=flat_o, in1=b1b[:], op=AL.add)
                        nc.scalar.activation(
                            h_buf[:, t * 128:(t + 1) * 128], flat_o,
                            AF.Relu)
                        shard2_row(t)
                    else:
                        nc.vector.tensor_tensor(
                            out=flat_o, in0=flat_o, in1=b2b[:], op=AL.add)
                        nc.vector.tensor_tensor(
                            out=flat_o, in0=flat_o,
                            in1=h_buf[:, t * 128:(t + 1) * 128], op=AL.add)
                        yt = sm.tile([128, 32], F32, tag="yt")
                        nc.vector.reduce_sum(
                            out=yt[:],
                            in_=o_t.rearrange("p h d -> p d h"),
                            axis=mybir.AxisListType.X,
                        )
                        nc.scalar.mul(yt[:], yt[:], 0.25)
                        nc.sync.dma_start(
                            out=y_out[t * 128:(t + 1) * 128, :],
                            in_=yt[:])

            for _rep in range(reps):
                if "notable1" not in ablate:
                    table1_build()
                er1_build()
                edge_phase(table1, er1, layer=1)

                # dummy row patch for shard2, then one AllGather
                nc.sync.dma_start(out=shard2[DUMLOC:DUMLOC + 1, :],
                                  in_=drow[:])
                if "nocoll" not in ablate:
                    nc.gpsimd.collective_compute(
                        "AllGather", AL.bypass,
                        replica_groups=[list(range(NCORES))],
                        ins=[shard2[:, :].opt()],
                        outs=[table2[:, :].opt()],
                    )

                edge_phase(table2, er2, layer=2)

    if gather_queues > 1:
        # Align each gather's SWDGE queue with its Tile-assigned DMASW sem
        # lane (proc 11..18) so a semaphore is only ever updated from the
        # queue it's locked to, while spreading desc-gen across queues.
        for blk in nc.m.functions[0].blocks:
            for inst in blk.instructions:
                if "DMAGather" in type(inst).__name__:
                    lane = inst.bass_scheduled_proc - 11
                    assert 0 <= lane < 8
                    inst.queue_num = lane % gather_queues

    nc.compile()
    return nc


# ------------------------------------------------------------------ kernel
_CACHE = {}


def _get_program_and_plan(src, dst):
    key = (src.tobytes()[:256], dst.tobytes()[:256], src.shape[0])
    if key not in _CACHE:
        plan = _plan(np.asarray(src), np.asarray(dst))
        prog = _build_program(plan)
        _CACHE[key] = (plan, prog)
    return _CACHE[key]


def _make_in_maps(plan, inputs):
    newid = plan["newid"]
    x = np.asarray(inputs["x"])

    def bd(v):  # [H,D] -> [128, 4] block-diag
        m = np.zeros((HD, H), np.float32)
        for h in range(H):
            m[h * D:(h + 1) * D, h] = v[h]
        return m

    W1 = np.asarray(inputs["W1"], np.float32)
    W2 = np.asarray(inputs["W2"], np.float32)
    al1 = np.asarray(inputs["al1"], np.float32)
    ar1 = np.asarray(inputs["ar1"], np.float32)
    al2 = np.asarray(inputs["al2"], np.float32)
    ar2 = np.asarray(inputs["ar2"], np.float32)

    w1ext = np.concatenate([W1, W1 @ bd(al1)], axis=1)       # [128, 132]
    w1ar = W1 @ bd(ar1)                                      # [128, 4]
    w2ext = np.concatenate([W2, W2 @ bd(al2), W2 @ bd(ar2)], axis=1)

    # permuted node table [NPRIME, 128]; pads zero
    xP = np.zeros((NPRIME, IN_F), np.float32)
    xP[newid] = x
    xT_full = np.ascontiguousarray(xP.T.astype(ml_dtypes.bfloat16))
    xPc = xP.reshape(NCORES, NPC, IN_F)

    dumrow = np.zeros((1, 256), np.float32)
    dumrow[0, 128:132] = EL_DUMMY

    common = {
        "xT_full": xT_full,
        "w1ext": w1ext.astype(ml_dtypes.bfloat16),
        "w1ar": w1ar.astype(ml_dtypes.bfloat16),
        "w2ext": w2ext.astype(ml_dtypes.bfloat16),
        "b1b": np.tile(np.asarray(inputs["b1"])[None, :], (128, 1)).astype(
            np.float32),
        "b2b": np.tile(np.asarray(inputs["b2"])[None, :], (128, 1)).astype(
            np.float32),
        "dumrow": dumrow.astype(ml_dtypes.bfloat16),
        "ident": np.eye(128, dtype=ml_dtypes.bfloat16),
    }
    in_maps = []
    for c in range(NCORES):
        m = dict(common)
        m["xT_own"] = np.ascontiguousarray(
            xPc[c].T.astype(ml_dtypes.bfloat16))
        m["idxlo"] = plan["idx_lo"][c]
        m["idxhi"] = plan["idx_hi"][c]
        in_maps.append(m)
    return in_maps


def _build_null_program(plan):
    """Same I/O signature, near-zero work — for dispatch-overhead timing."""
    SLO, SHI = plan["SLO"], plan["SHI"]
    nc = bacc.Bacc(None, target_bir_lowering=False, debug=False)
    nc.declare_dram_parameter("xT_full", [128, NPRIME], BF16, isOutput=False)
    nc.declare_dram_parameter("xT_own", [128, NPC], BF16, isOutput=False)
    nc.declare_dram_parameter("w1ext", [128, 132], BF16, isOutput=False)
    nc.declare_dram_parameter("w1ar", [128, 4], BF16, isOutput=False)
    nc.declare_dram_parameter("w2ext", [128, 136], BF16, isOutput=False)
    b1b_in = nc.declare_dram_parameter("b1b", [128, 128], F32, isOutput=False)
    nc.declare_dram_parameter("b2b", [128, 128], F32, isOutput=False)
    nc.declare_dram_parameter("dumrow", [1, 256], BF16, isOutput=False)
    nc.declare_dram_parameter("ident", [128, 128], BF16, isOutput=False)
    nc.declare_dram_parameter("idxlo", [128, SLO * 8], I16, isOutput=False)
    nc.declare_dram_parameter("idxhi", [128, SHI * 8], I16, isOutput=False)
    y_out = nc.declare_dram_parameter("y", [NPC, 32], F32, isOutput=True)
    with tile_mod.TileContext(nc) as tc:
        with tc.tile_pool(name="p", bufs=1) as p:
            t = p.tile([128, 32], F32)
            nc.sync.dma_start(out=t[:], in_=b1b_in[:, 0:32])
            nc.sync.dma_start(out=y_out[0:128, :], in_=t[:])
    nc.compile()
    return nc


def kernel(x, src, dst, W1, al1, ar1, b1, W2, al2, ar2, b2):
    src = np.asarray(src)
    dst = np.asarray(dst)
    plan, nc = _get_program_and_plan(src, dst)
    in_maps = _make_in_maps(plan, dict(
        x=x, W1=W1, al1=al1, ar1=ar1, b1=b1,
        W2=W2, al2=al2, ar2=ar2, b2=b2))

    from concourse.bass_utils import run_bass_kernel_spmd
    res = run_bass_kernel_spmd(nc, in_maps, core_ids=list(range(NCORES)))

    y = np.stack([res.results[c]["y"] for c in range(NCORES)])  # [8,NPC,32]
    out = y.reshape(NPRIME, 32)[plan["newid"]]
    return out.astype(np.float32)
